# revision 13
# baseline (speedup 1.0000x reference)
"""Causal self-attention (B=4, T=2048, C=1024, 16 heads) on 8 trn2 NeuronCores.

Sharding: tensor-parallel over heads (2 heads/core) for QKV + attention;
per-batch AllToAll reshards head-split -> token-split (256-token interleaved
slices per core) for the output projection, so 3 of 4 collectives and all
projections overlap with the next batch's compute.

Per-core pipeline (identical program on all cores; only the fed W_qkv
column-slice differs):
  x prep:  DRAM->DRAM casting DMA (f32->bf16), then hardware xbar
           DMA-transpose loads of xT tiles -- no PE/DVE involvement.
  stage 1: qT,kT [128ch x 2048tok] bf16 and token-major v per batch from
           xT @ W_qkv_slice.
  stage 2: per (window, jk-tile): both heads' S^T score matmuls issued
           back-to-back (they pack into disjoint PE row groups), causal mask
           applied by accumulating -30000 * staircase via a second matmul,
           one exp ACTIVATE per head-pair, AV accumulation with a ones
           column in v producing softmax denominators in PSUM row 64.
           Diagonal-band tiles skip their fully-masked column prefix in the
           score/exp/AV ops.  Denominators are DMA-reshaped to [128,4] so
           one DVE reciprocal covers a window (~115ns), broadcast on gpsimd.
  stage 3: after batch b's AllToAll: out[256-token slice] = y^T.T @ W_proj.
"""

import os
import numpy as np
from contextlib import ExitStack

from concourse import bass, bacc, mybir, tile
from concourse.bass_utils import run_bass_kernel_spmd

F32 = mybir.dt.float32
BF16 = mybir.dt.bfloat16

B, T, C = 4, 2048, 1024
H, D = 16, 64
NCORES = 8
HPC = H // NCORES            # heads per core = 2
QKC = HPC * D                # per-core q/k/v channels = 128
BT = B * T                   # 8192 tokens total
P = 128
TW = 512                     # q-window width
NW = T // TW                 # windows per batch = 4
NKT = T // P                 # kt tiles per batch = 16
TPB = T // NCORES            # tokens per core per batch after A2A = 256
KC = C // P                  # contraction chunks = 8
BIG = 30000.0                # additive causal mask magnitude

# knobs
XPREP = os.environ.get("KXPREP", "dma")   # "dma" (cast-DMA + xbar transpose) | "pe"
EXP_PAIR = os.environ.get("KEXPPAIR", "1") == "1"


def build() -> bass.Bass:
    nc = bacc.Bacc(num_devices=NCORES, target_bir_lowering=False)

    x_d = nc.dram_tensor("x", [BT, C], F32, kind="ExternalInput")
    wqkv_d = nc.dram_tensor("wqkv", [C, 3 * QKC], F32, kind="ExternalInput")
    wproj_d = nc.dram_tensor("wproj", [C, C], F32, kind="ExternalInput")
    out_d = nc.dram_tensor("out", [B * TPB, C], F32, kind="ExternalOutput")

    ident_d = nc.inline_tensor(np.eye(P, dtype=np.float32), name="ident")
    # mask: out[p, f] = -BIG * 1[p > f]  ==  (uneg.T @ ishift)[p, f]
    uneg_np = -BIG * np.triu(np.ones((P, P), dtype=np.float32))
    ishift_np = np.eye(P, k=-1, dtype=np.float32)
    uneg_d = nc.inline_tensor(uneg_np, name="uneg")
    ishift_d = nc.inline_tensor(ishift_np, name="ishift")

    with tile.TileContext(nc) as tc:
        with ExitStack() as ctx:
            # ---- persistent pools ----
            wq_pool = ctx.enter_context(tc.tile_pool(name="wq", bufs=1))
            wp_pool = ctx.enter_context(tc.tile_pool(name="wp", bufs=1))
            cst_pool = ctx.enter_context(tc.tile_pool(name="cst", bufs=1))
            dram = ctx.enter_context(tc.tile_pool(name="dram", bufs=1, space="DRAM"))

            wqkv_sb = wq_pool.tile([P, KC, 3 * QKC], BF16)
            wproj_sb = wp_pool.tile([P, KC, C], BF16)
            ident_bf = cst_pool.tile([P, P], BF16)
            uneg_bf = cst_pool.tile([P, P], BF16)
            ishift_bf = cst_pool.tile([P, P], BF16)

            x_bf = dram.tile([BT, C], BF16, tag="xbf")
            y_send = [
                dram.tile(
                    [NCORES, QKC, TPB], BF16, tag=f"ysend{b}", name=f"ysend{b}"
                )
                for b in range(B)
            ]
            y_recv = [
                dram.tile(
                    [NCORES, QKC, TPB], BF16, tag=f"yrecv{b}", name=f"yrecv{b}"
                )
                for b in range(B)
            ]

            # x f32 -> bf16 staging in DRAM (casting DMAs on SWDGE)
            if XPREP == "dma":
                for b in range(B):
                    for w in range(NW):
                        t0 = b * T + w * TW
                        nc.gpsimd.dma_start(
                            out=x_bf[t0 : t0 + TW, :], in_=x_d[t0 : t0 + TW, :]
                        )

            # ---- weights + constants: load f32, cast to bf16 ----
            with tc.tile_pool(name="stg", bufs=2) as stg:
                wqkv_st = stg.tile([P, KC, 3 * QKC], F32, tag="wst3", bufs=1)
                nc.scalar.dma_start(
                    out=wqkv_st[:],
                    in_=wqkv_d[:, :].rearrange("(k p) n -> p k n", p=P),
                )
                nc.vector.tensor_copy(wqkv_sb[:], wqkv_st[:])
                for kc in range(KC):
                    wproj_st = stg.tile([P, C], F32, tag="wpst")
                    nc.scalar.dma_start(
                        out=wproj_st[:], in_=wproj_d[kc * P : (kc + 1) * P, :]
                    )
                    nc.vector.tensor_copy(wproj_sb[:, kc, :], wproj_st[:])
                for dsrc, dst in (
                    (ident_d, ident_bf),
                    (uneg_d, uneg_bf),
                    (ishift_d, ishift_bf),
                ):
                    mst = stg.tile([P, P], F32, tag="mst")
                    nc.scalar.dma_start(out=mst[:], in_=dsrc[:, :])
                    nc.vector.tensor_copy(dst[:], mst[:])

            # ---- working pools ----
            xT_pool = ctx.enter_context(tc.tile_pool(name="xT", bufs=2))
            qkv_pool = ctx.enter_context(tc.tile_pool(name="qkv", bufs=2))
            qkvps = ctx.enter_context(
                tc.tile_pool(name="qkvps", bufs=2, space="PSUM")
            )
            pss = ctx.enter_context(tc.tile_pool(name="pss", bufs=2, space="PSUM"))
            psy = ctx.enter_context(tc.tile_pool(name="psy", bufs=1, space="PSUM"))
            pt_pool = ctx.enter_context(tc.tile_pool(name="pt", bufs=3))
            nrm_pool = ctx.enter_context(tc.tile_pool(name="nrm", bufs=2))
            yt_pool = ctx.enter_context(tc.tile_pool(name="yt", bufs=2))
            yr_pool = ctx.enter_context(tc.tile_pool(name="yr", bufs=2))
            ob_pool = ctx.enter_context(tc.tile_pool(name="ob", bufs=2))

            yu_saved: dict = {}

            def emit_norm_and_collective(bn: int):
                for w in range(NW):
                    yu = yu_saved.pop((bn, w))
                    den_sc = nrm_pool.tile([P, HPC, 4], F32, tag="densc", bufs=4)
                    for h in range(HPC):
                        nc.sync.dma_start(
                            out=den_sc[:, h, :], in_=yu[h][D : D + 1, :]
                        )
                    denr_sc = nrm_pool.tile([P, HPC, 4], F32, tag="denr", bufs=4)
                    nc.vector.reciprocal(denr_sc[:], den_sc[:])
                    bc_src = nrm_pool.tile([1, HPC, TW], F32, tag="bcsrc")
                    for h in range(HPC):
                        nc.sync.dma_start(
                            out=bc_src[0:1, h, :], in_=denr_sc[:, h, :]
                        )
                    for h in range(HPC):
                        bc_h = nrm_pool.tile([D, TW], F32, tag=f"bc{h}")
                        nc.gpsimd.partition_broadcast(bc_h[:], bc_src[0:1, h, :])
                        yt_h = yt_pool.tile([D, TW], BF16, tag=f"yt{h}", bufs=4)
                        nc.vector.tensor_mul(yt_h[:], yu[h][0:D, :], bc_h[:])
                        for s2 in range(2):
                            nc.sync.dma_start(
                                out=y_send[bn][2 * w + s2, h * D : (h + 1) * D, :],
                                in_=yt_h[:, s2 * TPB : (s2 + 1) * TPB],
                            )
                nc.gpsimd.collective_compute(
                    "AllToAll",
                    mybir.AluOpType.bypass,
                    replica_groups=[list(range(NCORES))],
                    ins=[y_send[bn].opt()],
                    outs=[y_recv[bn].opt()],
                )

            def emit_stage3(bn: int):
                yr = yr_pool.tile([P, NCORES, TPB], BF16, tag="yr")
                nc.sync.dma_start(
                    out=yr[:], in_=y_recv[bn][:, :, :].rearrange("j p t -> p j t")
                )
                for jt in range(TPB // P):
                    for half in range(C // TW):
                        ps_op = qkvps.tile([P, TW], F32, tag="ps1", name="ps_op")
                        for j in range(NCORES):
                            nc.tensor.matmul(
                                ps_op[:],
                                lhsT=yr[:, j, jt * P : (jt + 1) * P],
                                rhs=wproj_sb[:, j, half * TW : (half + 1) * TW],
                                start=(j == 0),
                                stop=(j == NCORES - 1),
                            )
                        ob = ob_pool.tile([P, TW], F32, tag="ob")
                        nc.vector.tensor_copy(ob[:], ps_op[:])
                        nc.sync.dma_start(
                            out=out_d[
                                bn * TPB + jt * P : bn * TPB + (jt + 1) * P,
                                half * TW : (half + 1) * TW,
                            ],
                            in_=ob[:],
                        )

            for b in range(B):
                qT_b = qkv_pool.tile([P, T], BF16, tag="qT")
                kT_b = qkv_pool.tile([P, T], BF16, tag="kT")
                v_b = qkv_pool.tile([P, NKT, HPC, D + 1], BF16, tag="v")
                nc.vector.memset(v_b[:, :, :, D : D + 1], 1.0)

                # ---- stage 1: qT, kT, v for batch b ----
                for w in range(NW):
                    t0 = b * T + w * TW
                    xTw = xT_pool.tile([P, KC, TW], BF16, tag="xT")
                    if XPREP == "dma":
                        for kc in range(KC):
                            nc.sync.dma_start(
                                out=xTw[:, kc, :],
                                in_=x_bf[t0 : t0 + TW, kc * P : (kc + 1) * P],
                                transpose=True,
                            )
                    else:
                        # fallback: f32 loads straight from x_d + DVE cast + PE transpose
                        xn = xT_pool.tile([P, TW // P, C], F32, tag="xn", bufs=2)
                        nc.sync.dma_start(
                            out=xn[:],
                            in_=x_d[t0 : t0 + TW, :].rearrange(
                                "(s p) c -> p s c", p=P
                            ),
                        )
                        xb = xT_pool.tile([P, TW // P, C], BF16, tag="xb", bufs=2)
                        nc.vector.tensor_copy(xb[:], xn[:])
                        for kc in range(KC):
                            ps_t = qkvps.tile([P, TW], BF16, tag="ps1", name="ps_t")
                            for s in range(TW // P):
                                nc.tensor.transpose(
                                    ps_t[:, s * P : (s + 1) * P],
                                    xb[:, s, kc * P : (kc + 1) * P],
                                    ident_bf[:],
                                )
                            nc.vector.tensor_copy(xTw[:, kc, :], ps_t[:])

                    for which, dst in ((0, qT_b), (1, kT_b)):
                        ps = qkvps.tile([P, TW], F32, tag="ps1", name="ps_qk")
                        for kc in range(KC):
                            nc.tensor.matmul(
                                ps[:],
                                lhsT=wqkv_sb[:, kc, which * QKC : (which + 1) * QKC],
                                rhs=xTw[:, kc, :],
                                start=(kc == 0),
                                stop=(kc == KC - 1),
                            )
                        nc.vector.tensor_copy(dst[:, w * TW : (w + 1) * TW], ps[:])

                    ps_vT = qkvps.tile([P, TW], F32, tag="ps1", name="ps_vT")
                    for kc in range(KC):
                        nc.tensor.matmul(
                            ps_vT[:],
                            lhsT=wqkv_sb[:, kc, 2 * QKC : 3 * QKC],
                            rhs=xTw[:, kc, :],
                            start=(kc == 0),
                            stop=(kc == KC - 1),
                        )
                    vT_sb = xT_pool.tile([P, TW], BF16, tag="vT", name="vT_sb")
                    nc.vector.tensor_copy(vT_sb[:], ps_vT[:])
                    ps_v = qkvps.tile([P, TW], BF16, tag="ps1", name="ps_v")
                    for s in range(TW // P):
                        nc.tensor.transpose(
                            ps_v[:, s * P : (s + 1) * P],
                            vT_sb[:, s * P : (s + 1) * P],
                            ident_bf[:],
                        )
                    jt0 = w * (TW // P)
                    nc.vector.tensor_copy(
                        v_b[:, jt0 : jt0 + TW // P, :, 0:D],
                        ps_v[:].rearrange("p (s h d) -> p s h d", s=TW // P, h=HPC),
                    )

                # ---- stage 2: attention for batch b ----
                for w in range(NW):
                    nkt = (w + 1) * (TW // P)
                    ps_y = [
                        psy.tile([D + 1, TW], F32, tag=f"psy{h}", name=f"ps_y{h}")
                        for h in range(HPC)
                    ]
                    # diagonal band first (full-width m=0 tile opens the chains)
                    jks = list(range(w * (TW // P), nkt)) + list(
                        range(0, w * (TW // P))
                    )
                    for ji, jk in enumerate(jks):
                        m = jk - w * (TW // P)
                        diag = m >= 0
                        c0 = P * m if diag else 0
                        ps_pair = pss.tile([P, HPC, TW], F32, tag="ps_s")
                        pt_pair = pt_pool.tile([P, HPC, TW], BF16, tag="pt")
                        for h in range(HPC):
                            nc.tensor.matmul(
                                ps_pair[:, h, c0:TW],
                                lhsT=kT_b[h * D : (h + 1) * D, jk * P : (jk + 1) * P],
                                rhs=qT_b[h * D : (h + 1) * D, w * TW + c0 : (w + 1) * TW],
                                start=True,
                                stop=not diag,
                            )
                        if diag:
                            for h in range(HPC):
                                nc.tensor.matmul(
                                    ps_pair[:, h, c0 : c0 + P],
                                    lhsT=uneg_bf[:],
                                    rhs=ishift_bf[:],
                                    start=False,
                                    stop=True,
                                )
                        if EXP_PAIR:
                            nc.scalar.activation(
                                pt_pair[:, :, c0:TW],
                                ps_pair[:, :, c0:TW],
                                mybir.ActivationFunctionType.Exp,
                                scale=1.0 / np.sqrt(D),
                            )
                        else:
                            for h in range(HPC):
                                nc.scalar.activation(
                                    pt_pair[:, h, c0:TW],
                                    ps_pair[:, h, c0:TW],
                                    mybir.ActivationFunctionType.Exp,
                                    scale=1.0 / np.sqrt(D),
                                )
                        for h in range(HPC):
                            nc.tensor.matmul(
                                ps_y[h][:, c0:TW],
                                lhsT=v_b[:, jk, h, :],
                                rhs=pt_pair[:, h, c0:TW],
                                start=(ji == 0),
                                stop=(ji == nkt - 1),
                            )

                    # ---- evacuate PSUM; normalization deferred one batch ----
                    yu = []
                    for h in range(HPC):
                        yu_h = yt_pool.tile([D + 1, TW], F32, tag=f"yu{h}", bufs=8)
                        nc.vector.tensor_copy(yu_h[:], ps_y[h][:])
                        yu.append(yu_h)
                    yu_saved[(b, w)] = yu

                # normalization + collective of the PREVIOUS batch: its latency
                # chain overlaps this batch's compute instead of gating it
                if b >= 1:
                    emit_norm_and_collective(b - 1)
            emit_norm_and_collective(B - 1)
            for b in range(B):
                emit_stage3(b)


    nc.finalize()
    return nc


_NC_CACHE: dict = {}


def _get_nc() -> bass.Bass:
    if "nc" not in _NC_CACHE:
        _NC_CACHE["nc"] = build()
    return _NC_CACHE["nc"]


def shard_inputs(x, W_qkv, W_proj):
    x = np.ascontiguousarray(np.asarray(x, dtype=np.float32).reshape(BT, C))
    W_qkv = np.asarray(W_qkv, dtype=np.float32)
    W_proj = np.ascontiguousarray(np.asarray(W_proj, dtype=np.float32))
    in_maps = []
    for c in range(NCORES):
        cols = slice(QKC * c, QKC * (c + 1))
        w_c = np.ascontiguousarray(
            np.concatenate(
                [W_qkv[:, cols], W_qkv[:, C:][:, cols], W_qkv[:, 2 * C :][:, cols]],
                axis=1,
            )
        )
        in_maps.append({"x": x, "wqkv": w_c, "wproj": W_proj})
    return in_maps


def assemble(res) -> np.ndarray:
    """res[c]["out"] rows are [b, i] = token (b, 256*c + i); reassemble."""
    arr = np.stack([np.asarray(res.results[c]["out"]) for c in range(NCORES)])
    arr = arr.reshape(NCORES, B, TPB, C).transpose(1, 0, 2, 3)
    return np.ascontiguousarray(arr.reshape(B, T, C)).astype(np.float32)


def run(in_maps, trace=False, **kwargs):
    return run_bass_kernel_spmd(
        _get_nc(), in_maps, core_ids=list(range(NCORES)), trace=trace, **kwargs
    )


def kernel(x, W_qkv, W_proj):
    res = run(shard_inputs(x, W_qkv, W_proj), trace=False)
    return assemble(res)


# revision 14
# speedup vs baseline: 1.0442x; 1.0442x over previous
"""Causal self-attention (B=4, T=2048, C=1024, 16 heads) on 8 trn2 NeuronCores.

Sharding: tensor-parallel over heads (2 heads/core) for QKV + attention;
per-batch AllToAll reshards head-split -> token-split (256-token interleaved
slices per core) for the output projection, so 3 of 4 collectives and all
projections overlap with the next batch's compute.

Per-core pipeline (identical program on all cores; only the fed W_qkv
column-slice differs):
  x prep:  DRAM->DRAM casting DMA (f32->bf16), then hardware xbar
           DMA-transpose loads of xT tiles -- no PE/DVE involvement.
  stage 1: qT,kT [128ch x 2048tok] bf16 and token-major v per batch from
           xT @ W_qkv_slice.
  stage 2: per (window, jk-tile): both heads' S^T score matmuls issued
           back-to-back (they pack into disjoint PE row groups), causal mask
           applied by accumulating -30000 * staircase via a second matmul,
           one exp ACTIVATE per head-pair, AV accumulation with a ones
           column in v producing softmax denominators in PSUM row 64.
           Diagonal-band tiles skip their fully-masked column prefix in the
           score/exp/AV ops.  Denominators are DMA-reshaped to [128,4] so
           one DVE reciprocal covers a window (~115ns), broadcast on gpsimd.
  stage 3: after batch b's AllToAll: out[256-token slice] = y^T.T @ W_proj.
"""

import os
import numpy as np
from contextlib import ExitStack

from concourse import bass, bacc, mybir, tile
from concourse.bass_utils import run_bass_kernel_spmd

F32 = mybir.dt.float32
BF16 = mybir.dt.bfloat16

B, T, C = 4, 2048, 1024
H, D = 16, 64
NCORES = 8
HPC = H // NCORES            # heads per core = 2
QKC = HPC * D                # per-core q/k/v channels = 128
BT = B * T                   # 8192 tokens total
P = 128
TW = 512                     # q-window width
NW = T // TW                 # windows per batch = 4
NKT = T // P                 # kt tiles per batch = 16
TPB = T // NCORES            # tokens per core per batch after A2A = 256
KC = C // P                  # contraction chunks = 8
BIG = 30000.0                # additive causal mask magnitude

# knobs
XPREP = os.environ.get("KXPREP", "dma")   # "dma" (cast-DMA + xbar transpose) | "pe"
EXP_PAIR = os.environ.get("KEXPPAIR", "1") == "1"


def build() -> bass.Bass:
    nc = bacc.Bacc(num_devices=NCORES, target_bir_lowering=False)

    x_d = nc.dram_tensor("x", [BT, C], F32, kind="ExternalInput")
    wqkv_d = nc.dram_tensor("wqkv", [C, 3 * QKC], F32, kind="ExternalInput")
    wproj_d = nc.dram_tensor("wproj", [C, C], F32, kind="ExternalInput")
    out_d = nc.dram_tensor("out", [B * TPB, C], F32, kind="ExternalOutput")

    ident_d = nc.inline_tensor(np.eye(P, dtype=np.float32), name="ident")
    # mask: out[p, f] = -BIG * 1[p > f]  ==  (uneg.T @ ishift)[p, f]
    uneg_np = -BIG * np.triu(np.ones((P, P), dtype=np.float32))
    ishift_np = np.eye(P, k=-1, dtype=np.float32)
    uneg_d = nc.inline_tensor(uneg_np, name="uneg")
    ishift_d = nc.inline_tensor(ishift_np, name="ishift")

    with tile.TileContext(nc) as tc:
        with ExitStack() as ctx:
            # ---- persistent pools ----
            wq_pool = ctx.enter_context(tc.tile_pool(name="wq", bufs=1))
            wp_pool = ctx.enter_context(tc.tile_pool(name="wp", bufs=1))
            cst_pool = ctx.enter_context(tc.tile_pool(name="cst", bufs=1))
            dram = ctx.enter_context(tc.tile_pool(name="dram", bufs=1, space="DRAM"))

            wqkv_sb = wq_pool.tile([P, KC, 3 * QKC], BF16)
            wproj_sb = wp_pool.tile([P, KC, C], BF16)
            ident_bf = cst_pool.tile([P, P], BF16)
            uneg_bf = cst_pool.tile([P, P], BF16)
            ishift_bf = cst_pool.tile([P, P], BF16)

            x_bf = dram.tile([BT, C], BF16, tag="xbf")
            y_send = [
                dram.tile(
                    [NCORES, QKC, TPB], BF16, tag=f"ysend{b}", name=f"ysend{b}"
                )
                for b in range(B)
            ]
            y_recv = [
                dram.tile(
                    [NCORES, QKC, TPB], BF16, tag=f"yrecv{b}", name=f"yrecv{b}"
                )
                for b in range(B)
            ]

            # x f32 -> bf16 staging in DRAM (casting DMAs on SWDGE)
            if XPREP == "dma":
                for b in range(B):
                    for w in range(NW):
                        t0 = b * T + w * TW
                        nc.gpsimd.dma_start(
                            out=x_bf[t0 : t0 + TW, :], in_=x_d[t0 : t0 + TW, :]
                        )

            # ---- weights + constants: load f32, cast to bf16 ----
            with tc.tile_pool(name="stg", bufs=2) as stg:
                wqkv_st = stg.tile([P, KC, 3 * QKC], F32, tag="wst3", bufs=1)
                nc.sync.dma_start(
                    out=wqkv_st[:],
                    in_=wqkv_d[:, :].rearrange("(k p) n -> p k n", p=P),
                )
                nc.vector.tensor_copy(wqkv_sb[:], wqkv_st[:])
                for kc in range(KC):
                    wproj_st = stg.tile([P, C], F32, tag="wpst")
                    nc.sync.dma_start(
                        out=wproj_st[:], in_=wproj_d[kc * P : (kc + 1) * P, :]
                    )
                    nc.vector.tensor_copy(wproj_sb[:, kc, :], wproj_st[:])
                for dsrc, dst in (
                    (ident_d, ident_bf),
                    (uneg_d, uneg_bf),
                    (ishift_d, ishift_bf),
                ):
                    mst = stg.tile([P, P], F32, tag="mst")
                    nc.sync.dma_start(out=mst[:], in_=dsrc[:, :])
                    nc.vector.tensor_copy(dst[:], mst[:])

            # ---- working pools ----
            xT_pool = ctx.enter_context(tc.tile_pool(name="xT", bufs=2))
            qkv_pool = ctx.enter_context(tc.tile_pool(name="qkv", bufs=2))
            qkvps = ctx.enter_context(
                tc.tile_pool(name="qkvps", bufs=2, space="PSUM")
            )
            pss = ctx.enter_context(tc.tile_pool(name="pss", bufs=2, space="PSUM"))
            psy = ctx.enter_context(tc.tile_pool(name="psy", bufs=1, space="PSUM"))
            pt_pool = ctx.enter_context(tc.tile_pool(name="pt", bufs=3))
            nrm_pool = ctx.enter_context(tc.tile_pool(name="nrm", bufs=2))
            yt_pool = ctx.enter_context(tc.tile_pool(name="yt", bufs=2))
            yr_pool = ctx.enter_context(tc.tile_pool(name="yr", bufs=2))
            ob_pool = ctx.enter_context(tc.tile_pool(name="ob", bufs=2))

            yu_saved: dict = {}

            def emit_norm_and_collective(bn: int):
                for w in range(NW):
                    yu = yu_saved.pop((bn, w))
                    den_sc = nrm_pool.tile([P, HPC, 4], F32, tag="densc", bufs=4)
                    for h in range(HPC):
                        nc.sync.dma_start(
                            out=den_sc[:, h, :], in_=yu[h][D : D + 1, :]
                        )
                    denr_sc = nrm_pool.tile([P, HPC, 4], F32, tag="denr", bufs=4)
                    nc.vector.reciprocal(denr_sc[:], den_sc[:])
                    bc_src = nrm_pool.tile([1, HPC, TW], F32, tag="bcsrc")
                    for h in range(HPC):
                        nc.sync.dma_start(
                            out=bc_src[0:1, h, :], in_=denr_sc[:, h, :]
                        )
                    for h in range(HPC):
                        bc_h = nrm_pool.tile([D, TW], F32, tag=f"bc{h}")
                        nc.gpsimd.partition_broadcast(bc_h[:], bc_src[0:1, h, :])
                        yt_h = yt_pool.tile([D, TW], BF16, tag=f"yt{h}", bufs=4)
                        nc.vector.tensor_mul(yt_h[:], yu[h][0:D, :], bc_h[:])
                        for s2 in range(2):
                            nc.sync.dma_start(
                                out=y_send[bn][2 * w + s2, h * D : (h + 1) * D, :],
                                in_=yt_h[:, s2 * TPB : (s2 + 1) * TPB],
                            )
                nc.gpsimd.collective_compute(
                    "AllToAll",
                    mybir.AluOpType.bypass,
                    replica_groups=[list(range(NCORES))],
                    ins=[y_send[bn].opt()],
                    outs=[y_recv[bn].opt()],
                )

            def emit_stage3(bn: int):
                yr = yr_pool.tile([P, NCORES, TPB], BF16, tag="yr")
                nc.sync.dma_start(
                    out=yr[:], in_=y_recv[bn][:, :, :].rearrange("j p t -> p j t")
                )
                for jt in range(TPB // P):
                    for half in range(C // TW):
                        ps_op = qkvps.tile([P, TW], F32, tag="ps1", name="ps_op")
                        for j in range(NCORES):
                            nc.tensor.matmul(
                                ps_op[:],
                                lhsT=yr[:, j, jt * P : (jt + 1) * P],
                                rhs=wproj_sb[:, j, half * TW : (half + 1) * TW],
                                start=(j == 0),
                                stop=(j == NCORES - 1),
                            )
                        ob = ob_pool.tile([P, TW], F32, tag="ob")
                        nc.vector.tensor_copy(ob[:], ps_op[:])
                        nc.sync.dma_start(
                            out=out_d[
                                bn * TPB + jt * P : bn * TPB + (jt + 1) * P,
                                half * TW : (half + 1) * TW,
                            ],
                            in_=ob[:],
                        )

            for b in range(B):
                qT_b = qkv_pool.tile([P, T], BF16, tag="qT")
                kT_b = qkv_pool.tile([P, T], BF16, tag="kT")
                v_b = qkv_pool.tile([P, NKT, HPC, D + 1], BF16, tag="v")
                nc.vector.memset(v_b[:, :, :, D : D + 1], 1.0)

                # ---- stage 1: qT, kT, v for batch b ----
                for w in range(NW):
                    t0 = b * T + w * TW
                    xTw = xT_pool.tile([P, KC, TW], BF16, tag="xT")
                    if XPREP == "dma":
                        for kc in range(KC):
                            nc.sync.dma_start(
                                out=xTw[:, kc, :],
                                in_=x_bf[t0 : t0 + TW, kc * P : (kc + 1) * P],
                                transpose=True,
                            )
                    else:
                        # fallback: f32 loads straight from x_d + DVE cast + PE transpose
                        xn = xT_pool.tile([P, TW // P, C], F32, tag="xn", bufs=2)
                        nc.sync.dma_start(
                            out=xn[:],
                            in_=x_d[t0 : t0 + TW, :].rearrange(
                                "(s p) c -> p s c", p=P
                            ),
                        )
                        xb = xT_pool.tile([P, TW // P, C], BF16, tag="xb", bufs=2)
                        nc.vector.tensor_copy(xb[:], xn[:])
                        for kc in range(KC):
                            ps_t = qkvps.tile([P, TW], BF16, tag="ps1", name="ps_t")
                            for s in range(TW // P):
                                nc.tensor.transpose(
                                    ps_t[:, s * P : (s + 1) * P],
                                    xb[:, s, kc * P : (kc + 1) * P],
                                    ident_bf[:],
                                )
                            nc.vector.tensor_copy(xTw[:, kc, :], ps_t[:])

                    for which, dst in ((0, qT_b), (1, kT_b)):
                        ps = qkvps.tile([P, TW], F32, tag="ps1", name="ps_qk")
                        for kc in range(KC):
                            nc.tensor.matmul(
                                ps[:],
                                lhsT=wqkv_sb[:, kc, which * QKC : (which + 1) * QKC],
                                rhs=xTw[:, kc, :],
                                start=(kc == 0),
                                stop=(kc == KC - 1),
                            )
                        nc.vector.tensor_copy(dst[:, w * TW : (w + 1) * TW], ps[:])

                    ps_vT = qkvps.tile([P, TW], F32, tag="ps1", name="ps_vT")
                    for kc in range(KC):
                        nc.tensor.matmul(
                            ps_vT[:],
                            lhsT=wqkv_sb[:, kc, 2 * QKC : 3 * QKC],
                            rhs=xTw[:, kc, :],
                            start=(kc == 0),
                            stop=(kc == KC - 1),
                        )
                    vT_sb = xT_pool.tile([P, TW], BF16, tag="vT", name="vT_sb")
                    nc.vector.tensor_copy(vT_sb[:], ps_vT[:])
                    ps_v = qkvps.tile([P, TW], BF16, tag="ps1", name="ps_v")
                    for s in range(TW // P):
                        nc.tensor.transpose(
                            ps_v[:, s * P : (s + 1) * P],
                            vT_sb[:, s * P : (s + 1) * P],
                            ident_bf[:],
                        )
                    jt0 = w * (TW // P)
                    nc.vector.tensor_copy(
                        v_b[:, jt0 : jt0 + TW // P, :, 0:D],
                        ps_v[:].rearrange("p (s h d) -> p s h d", s=TW // P, h=HPC),
                    )

                # ---- stage 2: attention for batch b ----
                for w in range(NW):
                    nkt = (w + 1) * (TW // P)
                    ps_y = [
                        psy.tile([D + 1, TW], F32, tag=f"psy{h}", name=f"ps_y{h}")
                        for h in range(HPC)
                    ]
                    # diagonal band first (full-width m=0 tile opens the chains)
                    jks = list(range(w * (TW // P), nkt)) + list(
                        range(0, w * (TW // P))
                    )
                    for ji, jk in enumerate(jks):
                        m = jk - w * (TW // P)
                        diag = m >= 0
                        c0 = P * m if diag else 0
                        ps_pair = pss.tile([P, HPC, TW], F32, tag="ps_s")
                        pt_pair = pt_pool.tile([P, HPC, TW], BF16, tag="pt")
                        for h in range(HPC):
                            nc.tensor.matmul(
                                ps_pair[:, h, c0:TW],
                                lhsT=kT_b[h * D : (h + 1) * D, jk * P : (jk + 1) * P],
                                rhs=qT_b[h * D : (h + 1) * D, w * TW + c0 : (w + 1) * TW],
                                start=True,
                                stop=not diag,
                            )
                        if diag:
                            for h in range(HPC):
                                nc.tensor.matmul(
                                    ps_pair[:, h, c0 : c0 + P],
                                    lhsT=uneg_bf[:],
                                    rhs=ishift_bf[:],
                                    start=False,
                                    stop=True,
                                )
                        if EXP_PAIR:
                            nc.scalar.activation(
                                pt_pair[:, :, c0:TW],
                                ps_pair[:, :, c0:TW],
                                mybir.ActivationFunctionType.Exp,
                                scale=1.0 / np.sqrt(D),
                            )
                        else:
                            for h in range(HPC):
                                nc.scalar.activation(
                                    pt_pair[:, h, c0:TW],
                                    ps_pair[:, h, c0:TW],
                                    mybir.ActivationFunctionType.Exp,
                                    scale=1.0 / np.sqrt(D),
                                )
                        for h in range(HPC):
                            nc.tensor.matmul(
                                ps_y[h][:, c0:TW],
                                lhsT=v_b[:, jk, h, :],
                                rhs=pt_pair[:, h, c0:TW],
                                start=(ji == 0),
                                stop=(ji == nkt - 1),
                            )

                    # ---- evacuate PSUM; normalization deferred one batch ----
                    yu = []
                    for h in range(HPC):
                        yu_h = yt_pool.tile([D + 1, TW], F32, tag=f"yu{h}", bufs=8)
                        nc.vector.tensor_copy(yu_h[:], ps_y[h][:])
                        yu.append(yu_h)
                    yu_saved[(b, w)] = yu

                # normalization + collective of the PREVIOUS batch: its latency
                # chain overlaps this batch's compute instead of gating it
                if b >= 1:
                    emit_norm_and_collective(b - 1)
            emit_norm_and_collective(B - 1)
            for b in range(B):
                emit_stage3(b)


    nc.finalize()
    return nc


_NC_CACHE: dict = {}


def _get_nc() -> bass.Bass:
    if "nc" not in _NC_CACHE:
        _NC_CACHE["nc"] = build()
    return _NC_CACHE["nc"]


def shard_inputs(x, W_qkv, W_proj):
    x = np.ascontiguousarray(np.asarray(x, dtype=np.float32).reshape(BT, C))
    W_qkv = np.asarray(W_qkv, dtype=np.float32)
    W_proj = np.ascontiguousarray(np.asarray(W_proj, dtype=np.float32))
    in_maps = []
    for c in range(NCORES):
        cols = slice(QKC * c, QKC * (c + 1))
        w_c = np.ascontiguousarray(
            np.concatenate(
                [W_qkv[:, cols], W_qkv[:, C:][:, cols], W_qkv[:, 2 * C :][:, cols]],
                axis=1,
            )
        )
        in_maps.append({"x": x, "wqkv": w_c, "wproj": W_proj})
    return in_maps


def assemble(res) -> np.ndarray:
    """res[c]["out"] rows are [b, i] = token (b, 256*c + i); reassemble."""
    arr = np.stack([np.asarray(res.results[c]["out"]) for c in range(NCORES)])
    arr = arr.reshape(NCORES, B, TPB, C).transpose(1, 0, 2, 3)
    return np.ascontiguousarray(arr.reshape(B, T, C)).astype(np.float32)


def run(in_maps, trace=False, **kwargs):
    return run_bass_kernel_spmd(
        _get_nc(), in_maps, core_ids=list(range(NCORES)), trace=trace, **kwargs
    )


def kernel(x, W_qkv, W_proj):
    res = run(shard_inputs(x, W_qkv, W_proj), trace=False)
    return assemble(res)
